# revision 9
# baseline (speedup 1.0000x reference)
"""Trainium2 Bass kernel for nn_JointPredReprModule (4-layer transformer w/ BatchNorm).

Sharding: data-parallel over batch (128 -> 16 per core x 8 cores).
Per-core activations are feature-major: x[d, token], token = b*128 + a*32 + s*16 + t
(s=0 obs slot, s=1 act slot; reference order is a*32 + 2t + s — mask is permuted to match).

v2 design notes (vs the earlier baseline):
- Residual stream kept in bf16 only (no fp32 shadow); BN stats accumulate in fp32.
- pos+seg biases folded on the host: obs slots pre-biased in the xinit upload, act
  slots get bias via an augmented one-hot matmul (extra t/a indicator rows).
- Attention: per-head Exp on the scalar engine with accum_out for the softmax
  denominators (no big vector reduce); one N=512 mask matmul per score bank.
- PSUM evacuation split across Scalar (ACT copy/relu/square) and Vector engines.
- Layer weights prefetched early; dummy matmuls keep the PE HAM-warm through the
  BatchNorm allreduce windows.
- Output staged compactly (obs slots only) and DMA'd per feature tile.
"""

import os
import numpy as np
import ml_dtypes

import concourse.bass as bass
import concourse.bacc as bacc
import concourse.mybir as mybir
import concourse.tile as tile
from concourse.bass_utils import run_bass_kernel_spmd

f32 = mybir.dt.float32
f16 = mybir.dt.float16
AX = mybir.AxisListType
OP = mybir.AluOpType
AF = mybir.ActivationFunctionType

L, B, A, D, H, ACTN = 16, 128, 4, 512, 8, 16
F = 2 * L * A          # 128 tokens per batch element
NCORES = 8
BL = B // NCORES       # 16 batch elems per core
T = BL * F             # 2048 tokens per core
DH = D // H            # 64
KT = D // 128          # 4 feature tiles
NCH = T // 512         # 4 token chunks of 512
MID = 4 * D            # 2048
MKT = MID // 128       # 16
EPS = 1e-5
NLAYERS = int(os.environ.get("KERNEL_NLAYERS", "4"))
MASKNEG = -240.0       # pre-scale; exp scale is 1/8 -> -30 post-scale
NTOT = float(B * F)    # global BN sample count
AUG = ACTN + L + A     # augmented one-hot rows (act + t-indicator + a-indicator)
NDUMMY = int(os.environ.get("KERNEL_NDUMMY", "0"))
# risky-construct toggles (default = safe/baseline-style)
_on = lambda name: os.environ.get(name, "0") == "1"
U_ACT_EVAC = _on("KERNEL_ACT_EVAC")   # psum evac on scalar engine
U_IDENT = _on("KERNEL_IDENT")         # ACT Identity with scale/bias APs
U_LNEXP = _on("KERNEL_LNEXP")         # rsqrt via ln+exp (single ACT table)
U_FASTMASK = _on("KERNEL_FASTMASK")   # one N=512 mask MM per score bank
U_HPS1 = _on("KERNEL_HPS1")           # single rearranged hT copy
AUGP = 64                             # padded augmented one-hot rows


def build_nc():
    nc = bacc.Bacc(None, target_bir_lowering=False, debug=False, num_devices=NCORES)

    xinit_d = nc.dram_tensor("xinit", [D, T], f16, kind="ExternalInput")
    onehot_d = nc.dram_tensor("onehot36", [AUGP, T // 2], f16, kind="ExternalInput")
    actW_d = nc.dram_tensor("actW36", [AUGP, D], f16, kind="ExternalInput")
    wq_d = nc.dram_tensor("wq", [4, D, D], f16, kind="ExternalInput")
    wk_d = nc.dram_tensor("wk", [4, D, D], f16, kind="ExternalInput")
    wv_d = nc.dram_tensor("wv", [4, D, D], f16, kind="ExternalInput")
    wc_d = nc.dram_tensor("wc", [4, D, D], f16, kind="ExternalInput")
    w1_d = nc.dram_tensor("w1", [4, D, MID], f16, kind="ExternalInput")
    w2_d = nc.dram_tensor("w2", [4, MID, D], f16, kind="ExternalInput")
    eye_d = nc.dram_tensor("eye", [128, 128], f16, kind="ExternalInput")
    maskp4_d = nc.dram_tensor("maskp4", [128, 512], f16, kind="ExternalInput")
    out_d = nc.dram_tensor("out", [D, T // 2], f32, kind="ExternalOutput")

    with tile.TileContext(nc) as tc:
        with (
            tc.tile_pool(name="sb", bufs=1) as sb,
            tc.tile_pool(name="ps", bufs=8, space="PSUM") as psp,
            tc.tile_pool(name="dram", bufs=2, space="DRAM") as dram,
        ):
            eye_sb = sb.tile([128, 128], f16, tag="eye", name="eye_sb")
            maskp4_sb = sb.tile([128, 512], f16, tag="maskp4", name="maskp4_sb")
            actW_sb = sb.tile([AUGP, D], f16, tag="actW", name="actW_sb")
            onehot_sb = sb.tile([AUGP, T // 2], f16, tag="onehot", name="onehot_sb")
            nc.sync.dma_start(eye_sb[:], eye_d[:])
            nc.sync.dma_start(maskp4_sb[:], maskp4_d[:])
            nc.sync.dma_start(actW_sb[:], actW_d[:])
            nc.sync.dma_start(onehot_sb[:], onehot_d[:])

            # residual stream (bf16), token = b*128 + a*32 + s*16 + t
            xb = []
            for k in range(KT):
                xk = sb.tile([128, T], f16, tag=f"xb{k}", name=f"xb{k}")
                xb.append(xk)
            xv = [x.rearrange("p (b a s t) -> p b a s t", b=BL, a=A, s=2, t=L)
                  for x in xb]

            # ---- weight prefetch machinery ----
            def alloc_wqkv(li):
                wq_sb = sb.tile([128, KT, D], f16, tag="wq", bufs=2, name=f"wq{li}")
                wk_sb = sb.tile([128, KT, D], f16, tag="wk", bufs=2, name=f"wk{li}")
                wv_sb = sb.tile([128, KT, D], f16, tag="wv", bufs=2, name=f"wv{li}")
                wc_sb = sb.tile([128, KT, D], f16, tag="wc", bufs=2, name=f"wc{li}")
                nc.sync.dma_start(wq_sb[:], wq_d[li].rearrange("(k p) m -> p k m", p=128))
                nc.sync.dma_start(wk_sb[:], wk_d[li].rearrange("(k p) m -> p k m", p=128))
                nc.sync.dma_start(wv_sb[:], wv_d[li].rearrange("(k p) m -> p k m", p=128))
                nc.sync.dma_start(wc_sb[:], wc_d[li].rearrange("(k p) m -> p k m", p=128))
                return (wq_sb, wk_sb, wv_sb, wc_sb)

            def alloc_w12(li):
                w1_sb = sb.tile([128, KT, MID], f16, tag="w1", bufs=1, name=f"w1_{li}")
                w2_sb = sb.tile([128, MKT, D], f16, tag="w2", bufs=1, name=f"w2_{li}")
                nc.sync.dma_start(w1_sb[:], w1_d[li].rearrange("(k p) m -> p k m", p=128))
                nc.sync.dma_start(w2_sb[:], w2_d[li].rearrange("(k p) m -> p k m", p=128))
                return w1_sb, w2_sb

            wqkv_next = alloc_wqkv(0)

            # ---- embedding ----
            for k in range(KT):
                nc.sync.dma_start(xb[k][:], xinit_d[k * 128:(k + 1) * 128, :])
            # act embedding (+ pos/seg bias via indicator rows)
            for m in range(KT):
                for cc in range(2):
                    aps = psp.tile([128, 512], f32, tag="ps", name="aps")
                    nc.tensor.matmul(
                        aps[:],
                        actW_sb[:, m * 128:(m + 1) * 128],
                        onehot_sb[:, cc * 512:(cc + 1) * 512],
                        start=True, stop=True,
                    )
                    nc.vector.tensor_copy(
                        xv[m][:, 8 * cc: 8 * cc + 8, :, 1, :], aps[:]
                    )

            # ---- per-layer pieces ----
            def qkv_chunk(li, c, w4, qT, kTt, vtok):
                wq_sb, wk_sb, wv_sb, _ = w4
                sl = slice(c * 512, (c + 1) * 512)
                for m in range(KT):
                    qps = psp.tile([128, 512], f32, tag="ps", name="qps")
                    for k in range(KT):
                        nc.tensor.matmul(
                            qps[:], wq_sb[:, k, m * 128:(m + 1) * 128],
                            xb[k][:, sl], start=(k == 0), stop=(k == KT - 1),
                        )
                    if U_ACT_EVAC:
                        nc.scalar.activation(qT[:, m, sl], qps[:], AF.Copy)
                    else:
                        nc.vector.tensor_copy(qT[:, m, sl], qps[:])
                for m in range(KT):
                    kps = psp.tile([128, 512], f32, tag="ps", name="kps")
                    for k in range(KT):
                        nc.tensor.matmul(
                            kps[:], wk_sb[:, k, m * 128:(m + 1) * 128],
                            xb[k][:, sl], start=(k == 0), stop=(k == KT - 1),
                        )
                    if U_ACT_EVAC:
                        nc.scalar.activation(kTt[:, m, sl], kps[:], AF.Copy)
                    else:
                        nc.vector.tensor_copy(kTt[:, m, sl], kps[:])
                for tt in range(4 * c, 4 * c + 4):
                    vps = psp.tile([128, 512], f32, tag="ps", name="vps")
                    for k in range(KT):
                        nc.tensor.matmul(
                            vps[:], xb[k][:, tt * 128:(tt + 1) * 128],
                            wv_sb[:, k, :], start=(k == 0), stop=(k == KT - 1),
                        )
                    nc.vector.tensor_copy(vtok[:, tt, :], vps[:])

            def attn_scores(b, qT, kTt, st):
                st[b] = []
                for q4 in range(2):
                    scps = psp.tile([128, 512], f32, tag="ps", name=f"scps{b}_{q4}")
                    if U_FASTMASK:
                        # mask first: one N=512 MM opens the bank (clears
                        # has_written bank-wide), scores accumulate after
                        nc.tensor.matmul(
                            scps[:], eye_sb[:], maskp4_sb[:],
                            start=True, stop=False, skip_group_check=True,
                        )
                        for hh in range(4):
                            h = q4 * 4 + hh
                            g, off = h // 2, (h % 2) * 64
                            nc.tensor.matmul(
                                scps[:, hh * 128:(hh + 1) * 128],
                                qT[off:off + 64, g, b * 128:(b + 1) * 128],
                                kTt[off:off + 64, g, b * 128:(b + 1) * 128],
                                start=False, stop=(hh == 3), skip_group_check=True,
                            )
                    else:
                        for hh in range(4):
                            h = q4 * 4 + hh
                            g, off = h // 2, (h % 2) * 64
                            nc.tensor.matmul(
                                scps[:, hh * 128:(hh + 1) * 128],
                                qT[off:off + 64, g, b * 128:(b + 1) * 128],
                                kTt[off:off + 64, g, b * 128:(b + 1) * 128],
                                start=True, stop=False,
                            )
                            nc.tensor.matmul(
                                scps[:, hh * 128:(hh + 1) * 128],
                                eye_sb[:], maskp4_sb[:, hh * 128:(hh + 1) * 128],
                                start=False, stop=True,
                            )
                    st[b].append(scps)

            def attn_soft(b, vtok, hT, st):
                E = sb.tile([128, H, 128], f16, tag="E", bufs=3, name=f"E{b}")
                ssum = sb.tile([128, H], f32, tag="ssum", bufs=4, name=f"ssum{b}")
                r = sb.tile([128, H], f32, tag="r", bufs=4, name=f"r{b}")
                for q4 in range(2):
                    scps = st[b][q4]
                    nc.scalar.activation(
                        E[:, q4 * 4:(q4 + 1) * 4, :], scps[:], AF.Exp, scale=0.125,
                    )
                nc.vector.tensor_reduce(ssum[:], E[:, :, :], AX.X, OP.add)
                nc.vector.reciprocal(r[:], ssum[:])
                at4 = []
                for q4 in range(2):
                    atps = psp.tile([128, 512], f32, tag="ps", name=f"atps{b}_{q4}")
                    for hh in range(4):
                        h = q4 * 4 + hh
                        diag = sb.tile([128, 128], f16, tag="diag", bufs=6,
                                       name=f"diag{b}_{h}")
                        if h % 2 == 1 and U_IDENT:
                            nc.scalar.activation(
                                diag[:], eye_sb[:], AF.Identity, scale=r[:, h:h + 1]
                            )
                        else:
                            nc.vector.tensor_scalar(
                                diag[:], eye_sb[:], r[:, h:h + 1], None, OP.mult
                            )
                        nc.tensor.matmul(
                            atps[:, hh * 128:(hh + 1) * 128],
                            E[:, h, :], diag[:], start=True, stop=True,
                        )
                    at = sb.tile([128, 512], f16, tag="at", bufs=4,
                                 name=f"at{b}_{q4}")
                    if q4 == 1 and U_ACT_EVAC:
                        nc.scalar.activation(at[:], atps[:], AF.Copy)
                    else:
                        nc.vector.tensor_copy(at[:], atps[:])
                    at4.append(at)
                hps = psp.tile([128, 512], f32, tag="ps", name=f"hps{b}")
                for h in range(H):
                    g, off = h // 2, (h % 2) * 64
                    nc.tensor.matmul(
                        hps[off:off + 64, g * 128:(g + 1) * 128],
                        vtok[:, b, h * 64:(h + 1) * 64],
                        at4[h // 4][:, (h % 4) * 128:(h % 4 + 1) * 128],
                        start=True, stop=True, tile_position=(0, off),
                    )
                if U_HPS1:
                    nc.vector.tensor_copy(
                        hT[:, :, b * 128:(b + 1) * 128],
                        hps.rearrange("p (g t) -> p g t", g=KT),
                    )
                else:
                    for g in range(KT):
                        nc.vector.tensor_copy(
                            hT[:, g, b * 128:(b + 1) * 128],
                            hps[:, g * 128:(g + 1) * 128],
                        )

            def outproj_chunk(li, c, w4, hT, asum, asq):
                wc_sb = w4[3]
                sl = slice(c * 512, (c + 1) * 512)
                for m in range(KT):
                    cps = psp.tile([128, 512], f32, tag="ps", name="cps")
                    for k in range(KT):
                        nc.tensor.matmul(
                            cps[:], wc_sb[:, k, m * 128:(m + 1) * 128],
                            hT[:, k, sl], start=(k == 0), stop=(k == KT - 1),
                        )
                    nc.vector.scalar_tensor_tensor(
                        xb[m][:, sl], cps[:], 1.0, xb[m][:, sl],
                        OP.mult, OP.add, accum_out=asum[:, m, c:c + 1],
                    )
                    scrap = sb.tile([128, 512], f16, tag="scrap", bufs=2,
                                    name="scrap")
                    nc.vector.scalar_tensor_tensor(
                        scrap[:], xb[m][:, sl], 1.0, xb[m][:, sl],
                        OP.mult, OP.mult, accum_out=asq[:, m, c:c + 1],
                    )

            def ffn_chunk(li, c, w1_sb, w2_sb, asum, asq):
                sl = slice(c * 512, (c + 1) * 512)
                midt = sb.tile([128, MKT, 512], f16, tag="mid", bufs=1,
                               name=f"mid{li}_{c}")
                for mm in range(MKT):
                    mps = psp.tile([128, 512], f32, tag="ps", name="mps")
                    for k in range(KT):
                        nc.tensor.matmul(
                            mps[:], w1_sb[:, k, mm * 128:(mm + 1) * 128],
                            xb[k][:, sl], start=(k == 0), stop=(k == KT - 1),
                        )
                    if U_ACT_EVAC:
                        nc.scalar.activation(midt[:, mm, :], mps[:], AF.Relu)
                    else:
                        nc.vector.tensor_scalar(
                            midt[:, mm, :], mps[:], 0.0, None, OP.max
                        )
                for m in range(KT):
                    ops = psp.tile([128, 512], f32, tag="ps", name="ops")
                    for k in range(MKT):
                        nc.tensor.matmul(
                            ops[:], w2_sb[:, k, m * 128:(m + 1) * 128],
                            midt[:, k, :], start=(k == 0), stop=(k == MKT - 1),
                        )
                    nc.vector.scalar_tensor_tensor(
                        xb[m][:, sl], ops[:], 1.0, xb[m][:, sl],
                        OP.mult, OP.add, accum_out=asum[:, m, c:c + 1],
                    )
                    scrap = sb.tile([128, 512], f16, tag="scrap", bufs=2,
                                    name="scrap")
                    nc.vector.scalar_tensor_tensor(
                        scrap[:], xb[m][:, sl], 1.0, xb[m][:, sl],
                        OP.mult, OP.mult, accum_out=asq[:, m, c:c + 1],
                    )

            def bn(li, bnidx, asum, asq, last=False):
                red = sb.tile([128, 2 * KT], f32, tag="red", bufs=2,
                              name=f"red{li}_{bnidx}")
                for m in range(KT):
                    nc.vector.tensor_reduce(red[:, 2 * m:2 * m + 1],
                                            asum[:, m, :], AX.X, OP.add)
                    nc.vector.tensor_reduce(red[:, 2 * m + 1:2 * m + 2],
                                            asq[:, m, :], AX.X, OP.add)
                cin = dram.tile([128, 2 * KT], f32, tag="cin",
                                name=f"cin{li}_{bnidx}")
                cout = dram.tile([128, 2 * KT], f32, tag="cout",
                                 name=f"cout{li}_{bnidx}")
                nc.sync.dma_start(cin[:], red[:])
                nc.gpsimd.collective_compute(
                    "AllReduce", OP.add,
                    replica_groups=[list(range(NCORES))],
                    ins=[cin.opt()], outs=[cout.opt()],
                )
                # keep the PE HAM-warm while the allreduce is in flight
                for dnum in range(NDUMMY):
                    dps = psp.tile([128, 512], f32, tag="ps",
                                   name=f"dps{li}_{bnidx}_{dnum}")
                    nc.tensor.matmul(dps[:], eye_sb[:], maskp4_sb[:],
                                     start=True, stop=True)
                redg = sb.tile([128, 2 * KT], f32, tag="redg", bufs=2,
                               name=f"redg{li}_{bnidx}")
                nc.sync.dma_start(redg[:], cout[:])
                redv = redg.rearrange("p (m two) -> p m two", two=2)
                stat = sb.tile([128, 4, KT], f32, tag="stat", bufs=2,
                               name=f"stat{li}_{bnidx}")
                a_sb = sb.tile([128, KT], f32, tag="a_sb", bufs=2,
                               name=f"a{li}_{bnidx}")
                bneg = sb.tile([128, KT], f32, tag="bneg", bufs=2,
                               name=f"bneg{li}_{bnidx}")
                nc.vector.tensor_scalar(stat[:, 0, :], redv[:, :, 0],
                                        1.0 / NTOT, None, OP.mult)
                nc.vector.tensor_scalar(stat[:, 1, :], redv[:, :, 1],
                                        1.0 / NTOT, None, OP.mult)
                nc.vector.tensor_mul(stat[:, 2, :], stat[:, 0, :], stat[:, 0, :])
                nc.vector.tensor_sub(stat[:, 3, :], stat[:, 1, :], stat[:, 2, :])
                nc.vector.tensor_scalar(stat[:, 3, :], stat[:, 3, :], EPS,
                                        None, OP.add)
                if U_LNEXP:
                    # a = 1/sqrt(var) = exp(-0.5*ln(var)); ln+exp share one ACT
                    # table set with the attention Exp -> no table reloads.
                    nc.scalar.activation(stat[:, 2, :], stat[:, 3, :], AF.Ln)
                    nc.scalar.activation(a_sb[:], stat[:, 2, :], AF.Exp, scale=-0.5)
                else:
                    nc.scalar.activation(stat[:, 2, :], stat[:, 3, :], AF.Sqrt)
                    nc.vector.reciprocal(a_sb[:], stat[:, 2, :])
                nc.vector.scalar_tensor_tensor(bneg[:], stat[:, 0, :], -1.0,
                                               a_sb[:], OP.mult, OP.mult)
                if last:
                    outst = []
                    for m in range(KT):
                        ot = sb.tile([128, T // 2], f32, tag=f"outst{m}",
                                     name=f"outst{m}")
                        outst.append(ot)
                    for c in range(NCH):
                        for m in range(KT):
                            src = xv[m][:, 4 * c:4 * c + 4, :, 0, :]
                            dst = outst[m].rearrange(
                                "p (b a t) -> p b a t", b=BL, a=A
                            )[:, 4 * c:4 * c + 4, :, :]
                            if m % 2 == 1 and U_IDENT:
                                nc.scalar.activation(
                                    dst, src, AF.Identity,
                                    bias=bneg[:, m:m + 1], scale=a_sb[:, m:m + 1],
                                )
                            else:
                                nc.vector.tensor_scalar(
                                    dst, src, a_sb[:, m:m + 1],
                                    bneg[:, m:m + 1], OP.mult, OP.add,
                                )
                    for m in range(KT):
                        nc.sync.dma_start(out_d[m * 128:(m + 1) * 128, :],
                                          outst[m][:])
                else:
                    for c in range(NCH):
                        for m in range(KT):
                            sl = slice(c * 512, (c + 1) * 512)
                            if m % 2 == 1 and U_IDENT:
                                nc.scalar.activation(
                                    xb[m][:, sl], xb[m][:, sl], AF.Identity,
                                    bias=bneg[:, m:m + 1], scale=a_sb[:, m:m + 1],
                                )
                            else:
                                nc.vector.tensor_scalar(
                                    xb[m][:, sl], xb[m][:, sl],
                                    a_sb[:, m:m + 1], bneg[:, m:m + 1],
                                    OP.mult, OP.add,
                                )

            # ---- layers ----
            for li in range(NLAYERS):
                w4 = wqkv_next
                w1_sb, w2_sb = alloc_w12(li)
                qT = sb.tile([128, KT, T], f16, tag="qT", name=f"qT{li}")
                kTt = sb.tile([128, KT, T], f16, tag="kT", name=f"kT{li}")
                vtok = sb.tile([128, BL, D], f16, tag="vtok", name=f"vtok{li}")
                hT = sb.tile([128, KT, T], f16, tag="hT", name=f"hT{li}")
                asum1 = sb.tile([128, KT, NCH], f32, tag="asum", bufs=2,
                                name=f"asum1_{li}")
                asq1 = sb.tile([128, KT, NCH], f32, tag="asq", bufs=2,
                               name=f"asq1_{li}")
                asum2 = sb.tile([128, KT, NCH], f32, tag="asum", bufs=2,
                                name=f"asum2_{li}")
                asq2 = sb.tile([128, KT, NCH], f32, tag="asq", bufs=2,
                               name=f"asq2_{li}")

                st = {}
                qkv_chunk(li, 0, w4, qT, kTt, vtok)
                qkv_chunk(li, 1, w4, qT, kTt, vtok)
                attn_scores(0, qT, kTt, st)
                for b in range(BL):
                    if b < BL - 1:
                        attn_scores(b + 1, qT, kTt, st)
                    attn_soft(b, vtok, hT, st)
                    if b == 3:
                        qkv_chunk(li, 2, w4, qT, kTt, vtok)
                    elif b == 7:
                        qkv_chunk(li, 3, w4, qT, kTt, vtok)
                    elif b == 11:
                        outproj_chunk(li, 0, w4, hT, asum1, asq1)
                outproj_chunk(li, 1, w4, hT, asum1, asq1)
                outproj_chunk(li, 2, w4, hT, asum1, asq1)
                outproj_chunk(li, 3, w4, hT, asum1, asq1)
                if li + 1 < NLAYERS:
                    wqkv_next = alloc_wqkv(li + 1)
                bn(li, 1, asum1, asq1)
                for c in range(NCH):
                    ffn_chunk(li, c, w1_sb, w2_sb, asum2, asq2)
                bn(li, 2, asum2, asq2, last=(li == NLAYERS - 1))
    return nc


def _prep_inputs(inputs):
    """Host-side sharding/layout prep. Returns per-core in_maps."""
    obs = np.asarray(inputs["obs_emb"], np.float32)        # [L,B,A,D]
    onehot = np.asarray(inputs["act_onehot"], np.float32)  # [L,B,A,ACTN]
    actW = np.asarray(inputs["act_W"], np.float32)         # [ACTN,D]
    pos = np.asarray(inputs["pos"], np.float32)            # [L,D]
    seg = np.asarray(inputs["seg_emb"], np.float32)        # [A,D]
    wq = np.ascontiguousarray(np.asarray(inputs["Wq"], np.float32)).astype(np.float16)
    wk = np.ascontiguousarray(np.asarray(inputs["Wk"], np.float32)).astype(np.float16)
    wv = np.ascontiguousarray(np.asarray(inputs["Wv"], np.float32)).astype(np.float16)
    wc = np.ascontiguousarray(np.asarray(inputs["Wc"], np.float32)).astype(np.float16)
    w1 = np.ascontiguousarray(np.asarray(inputs["W1"], np.float32)).astype(np.float16)
    w2 = np.ascontiguousarray(np.asarray(inputs["W2"], np.float32)).astype(np.float16)
    mask = np.asarray(inputs["mask"])                      # [F,F] bool

    eye = np.eye(128, dtype=np.float32).astype(np.float16)
    # permute mask from reference order (a*32 + 2t + s) to ours (a*32 + s*16 + t)
    perm = np.array([a * 32 + 2 * t + s
                     for a in range(A) for s in range(2) for t in range(L)])
    mp = mask[perm][:, perm]
    maskp = np.where(mp, 0.0, MASKNEG).astype(np.float32)
    maskp4 = np.ascontiguousarray(
        np.concatenate([maskp] * 4, axis=1)).astype(np.float16)

    # augmented act weights: [actW; pos; seg] so the act matmul adds pos+seg
    actW36 = np.concatenate(
        [actW, pos, seg, np.zeros((AUGP - AUG, D), np.float32)],
        axis=0).astype(np.float16)
    # indicator rows for act tokens in (b, a, t) order
    idx = np.arange(T // 2)
    t_of, a_of = idx % L, (idx // L) % A
    tind = (np.arange(L)[:, None] == t_of[None, :]).astype(np.float32)
    aind = (np.arange(A)[:, None] == a_of[None, :]).astype(np.float32)

    obsb = obs + pos[:, None, None, :] + seg[None, None, :, :]  # [L,B,A,D]

    in_maps = []
    for c in range(NCORES):
        bs = slice(c * BL, (c + 1) * BL)
        o = obsb[:, bs].transpose(3, 1, 2, 0)                  # [D,BL,A,L]
        xinit = np.zeros((D, BL, A, 2, L), np.float32)
        xinit[:, :, :, 0, :] = o
        xinit = np.ascontiguousarray(
            xinit.reshape(D, T)).astype(np.float16)
        oh = onehot[:, bs].transpose(3, 1, 2, 0).reshape(ACTN, T // 2)
        onehot36 = np.ascontiguousarray(np.concatenate(
            [oh, tind, aind, np.zeros((AUGP - AUG, T // 2), np.float32)],
            axis=0)).astype(np.float16)
        in_maps.append({
            "xinit": xinit, "onehot36": onehot36, "actW36": actW36,
            "wq": wq, "wk": wk, "wv": wv, "wc": wc, "w1": w1, "w2": w2,
            "eye": eye, "maskp4": maskp4,
        })
    return in_maps


def run_impl(inputs, trace=False):
    in_maps = _prep_inputs(inputs)
    nc = build_nc()
    nc.compile()
    res = run_bass_kernel_spmd(nc, in_maps, list(range(NCORES)), trace=trace)
    outs = []
    for c in range(NCORES):
        o = res.results[c]["out"]                     # [512, 1024]
        outs.append(o.reshape(D, BL, 2 * L * A // 2).transpose(1, 2, 0))
    full = np.concatenate(outs, axis=0)               # [B, 64, 512]
    return np.ascontiguousarray(full.astype(np.float32)), res


def kernel(**inputs) -> np.ndarray:
    out, _ = run_impl(inputs, trace=False)
    return out


# revision 10
# speedup vs baseline: 1.0154x; 1.0154x over previous
"""Trainium2 Bass kernel for nn_JointPredReprModule (4-layer transformer w/ BatchNorm).

Sharding: data-parallel over batch (128 -> 16 per core x 8 cores).
Per-core activations are feature-major: x[d, token], token = b*128 + a*32 + s*16 + t
(s=0 obs slot, s=1 act slot; reference order is a*32 + 2t + s — mask is permuted to match).

v2 design notes (vs the earlier baseline):
- Residual stream kept in bf16 only (no fp32 shadow); BN stats accumulate in fp32.
- pos+seg biases folded on the host: obs slots pre-biased in the xinit upload, act
  slots get bias via an augmented one-hot matmul (extra t/a indicator rows).
- Attention: per-head Exp on the scalar engine with accum_out for the softmax
  denominators (no big vector reduce); one N=512 mask matmul per score bank.
- PSUM evacuation split across Scalar (ACT copy/relu/square) and Vector engines.
- Layer weights prefetched early; dummy matmuls keep the PE HAM-warm through the
  BatchNorm allreduce windows.
- Output staged compactly (obs slots only) and DMA'd per feature tile.
"""

import os
import numpy as np
import ml_dtypes

import concourse.bass as bass
import concourse.bacc as bacc
import concourse.mybir as mybir
import concourse.tile as tile
from concourse.bass_utils import run_bass_kernel_spmd

f32 = mybir.dt.float32
f16 = mybir.dt.float16
AX = mybir.AxisListType
OP = mybir.AluOpType
AF = mybir.ActivationFunctionType

L, B, A, D, H, ACTN = 16, 128, 4, 512, 8, 16
F = 2 * L * A          # 128 tokens per batch element
NCORES = 8
BL = B // NCORES       # 16 batch elems per core
T = BL * F             # 2048 tokens per core
DH = D // H            # 64
KT = D // 128          # 4 feature tiles
NCH = T // 512         # 4 token chunks of 512
MID = 4 * D            # 2048
MKT = MID // 128       # 16
EPS = 1e-5
NLAYERS = int(os.environ.get("KERNEL_NLAYERS", "4"))
MASKNEG = -240.0       # pre-scale; exp scale is 1/8 -> -30 post-scale
NTOT = float(B * F)    # global BN sample count
AUG = ACTN + L + A     # augmented one-hot rows (act + t-indicator + a-indicator)
NDUMMY = int(os.environ.get("KERNEL_NDUMMY", "0"))
# risky-construct toggles (default = safe/baseline-style)
_on = lambda name: os.environ.get(name, "0") == "1"
U_ACT_EVAC = _on("KERNEL_ACT_EVAC")   # psum evac on scalar engine
U_IDENT = _on("KERNEL_IDENT")         # ACT Identity with scale/bias APs
U_LNEXP = _on("KERNEL_LNEXP")         # rsqrt via ln+exp (single ACT table)
U_FASTMASK = _on("KERNEL_FASTMASK")   # one N=512 mask MM per score bank
U_HPS1 = _on("KERNEL_HPS1")           # single rearranged hT copy
U_ACT_ACCUM = _on("KERNEL_ACT_ACCUM") # accum_out on scalar-engine activation
AUGP = 64                             # padded augmented one-hot rows


def build_nc():
    nc = bacc.Bacc(None, target_bir_lowering=False, debug=False, num_devices=NCORES)

    xinit_d = nc.dram_tensor("xinit", [D, T], f16, kind="ExternalInput")
    onehot_d = nc.dram_tensor("onehot36", [AUGP, T // 2], f16, kind="ExternalInput")
    actW_d = nc.dram_tensor("actW36", [AUGP, D], f16, kind="ExternalInput")
    wq_d = nc.dram_tensor("wq", [4, D, D], f16, kind="ExternalInput")
    wk_d = nc.dram_tensor("wk", [4, D, D], f16, kind="ExternalInput")
    wv_d = nc.dram_tensor("wv", [4, D, D], f16, kind="ExternalInput")
    wc_d = nc.dram_tensor("wc", [4, D, D], f16, kind="ExternalInput")
    w1_d = nc.dram_tensor("w1", [4, D, MID], f16, kind="ExternalInput")
    w2_d = nc.dram_tensor("w2", [4, MID, D], f16, kind="ExternalInput")
    eye_d = nc.dram_tensor("eye", [128, 128], f16, kind="ExternalInput")
    maskp4_d = nc.dram_tensor("maskp4", [128, 512], f16, kind="ExternalInput")
    out_d = nc.dram_tensor("out", [D, T // 2], f32, kind="ExternalOutput")

    with tile.TileContext(nc) as tc:
        with (
            tc.tile_pool(name="sb", bufs=1) as sb,
            tc.tile_pool(name="ps", bufs=8, space="PSUM") as psp,
            tc.tile_pool(name="dram", bufs=2, space="DRAM") as dram,
        ):
            eye_sb = sb.tile([128, 128], f16, tag="eye", name="eye_sb")
            maskp4_sb = sb.tile([128, 512], f16, tag="maskp4", name="maskp4_sb")
            actW_sb = sb.tile([AUGP, D], f16, tag="actW", name="actW_sb")
            onehot_sb = sb.tile([AUGP, T // 2], f16, tag="onehot", name="onehot_sb")
            nc.sync.dma_start(eye_sb[:], eye_d[:])
            nc.sync.dma_start(maskp4_sb[:], maskp4_d[:])
            nc.sync.dma_start(actW_sb[:], actW_d[:])
            nc.sync.dma_start(onehot_sb[:], onehot_d[:])

            # residual stream (bf16), token = b*128 + a*32 + s*16 + t
            xb = []
            for k in range(KT):
                xk = sb.tile([128, T], f16, tag=f"xb{k}", name=f"xb{k}")
                xb.append(xk)
            xv = [x.rearrange("p (b a s t) -> p b a s t", b=BL, a=A, s=2, t=L)
                  for x in xb]

            # ---- weight prefetch machinery ----
            def alloc_wqkv(li):
                wq_sb = sb.tile([128, KT, D], f16, tag="wq", bufs=2, name=f"wq{li}")
                wk_sb = sb.tile([128, KT, D], f16, tag="wk", bufs=2, name=f"wk{li}")
                wv_sb = sb.tile([128, KT, D], f16, tag="wv", bufs=2, name=f"wv{li}")
                wc_sb = sb.tile([128, KT, D], f16, tag="wc", bufs=2, name=f"wc{li}")
                nc.sync.dma_start(wq_sb[:], wq_d[li].rearrange("(k p) m -> p k m", p=128))
                nc.sync.dma_start(wk_sb[:], wk_d[li].rearrange("(k p) m -> p k m", p=128))
                nc.sync.dma_start(wv_sb[:], wv_d[li].rearrange("(k p) m -> p k m", p=128))
                nc.sync.dma_start(wc_sb[:], wc_d[li].rearrange("(k p) m -> p k m", p=128))
                return (wq_sb, wk_sb, wv_sb, wc_sb)

            def alloc_w12(li):
                w1_sb = sb.tile([128, KT, MID], f16, tag="w1", bufs=1, name=f"w1_{li}")
                w2_sb = sb.tile([128, MKT, D], f16, tag="w2", bufs=1, name=f"w2_{li}")
                nc.sync.dma_start(w1_sb[:], w1_d[li].rearrange("(k p) m -> p k m", p=128))
                nc.sync.dma_start(w2_sb[:], w2_d[li].rearrange("(k p) m -> p k m", p=128))
                return w1_sb, w2_sb

            wqkv_next = alloc_wqkv(0)

            # ---- embedding ----
            for k in range(KT):
                nc.sync.dma_start(xb[k][:], xinit_d[k * 128:(k + 1) * 128, :])
            # act embedding (+ pos/seg bias via indicator rows)
            for m in range(KT):
                for cc in range(2):
                    aps = psp.tile([128, 512], f32, tag="ps", name="aps")
                    nc.tensor.matmul(
                        aps[:],
                        actW_sb[:, m * 128:(m + 1) * 128],
                        onehot_sb[:, cc * 512:(cc + 1) * 512],
                        start=True, stop=True,
                    )
                    nc.vector.tensor_copy(
                        xv[m][:, 8 * cc: 8 * cc + 8, :, 1, :], aps[:]
                    )

            # ---- per-layer pieces ----
            def qkv_chunk(li, c, w4, qT, kTt, vtok):
                wq_sb, wk_sb, wv_sb, _ = w4
                sl = slice(c * 512, (c + 1) * 512)
                for m in range(KT):
                    qps = psp.tile([128, 512], f32, tag="ps", name="qps")
                    for k in range(KT):
                        nc.tensor.matmul(
                            qps[:], wq_sb[:, k, m * 128:(m + 1) * 128],
                            xb[k][:, sl], start=(k == 0), stop=(k == KT - 1),
                        )
                    if U_ACT_EVAC:
                        nc.scalar.activation(qT[:, m, sl], qps[:], AF.Copy)
                    else:
                        nc.vector.tensor_copy(qT[:, m, sl], qps[:])
                for m in range(KT):
                    kps = psp.tile([128, 512], f32, tag="ps", name="kps")
                    for k in range(KT):
                        nc.tensor.matmul(
                            kps[:], wk_sb[:, k, m * 128:(m + 1) * 128],
                            xb[k][:, sl], start=(k == 0), stop=(k == KT - 1),
                        )
                    if U_ACT_EVAC:
                        nc.scalar.activation(kTt[:, m, sl], kps[:], AF.Copy)
                    else:
                        nc.vector.tensor_copy(kTt[:, m, sl], kps[:])
                for tt in range(4 * c, 4 * c + 4):
                    vps = psp.tile([128, 512], f32, tag="ps", name="vps")
                    for k in range(KT):
                        nc.tensor.matmul(
                            vps[:], xb[k][:, tt * 128:(tt + 1) * 128],
                            wv_sb[:, k, :], start=(k == 0), stop=(k == KT - 1),
                        )
                    nc.vector.tensor_copy(vtok[:, tt, :], vps[:])

            def attn_scores(b, qT, kTt, st):
                st[b] = []
                for q4 in range(2):
                    scps = psp.tile([128, 512], f32, tag="ps", name=f"scps{b}_{q4}")
                    if U_FASTMASK:
                        # mask first: one N=512 MM opens the bank (clears
                        # has_written bank-wide), scores accumulate after
                        nc.tensor.matmul(
                            scps[:], eye_sb[:], maskp4_sb[:],
                            start=True, stop=False, skip_group_check=True,
                        )
                        for hh in range(4):
                            h = q4 * 4 + hh
                            g, off = h // 2, (h % 2) * 64
                            nc.tensor.matmul(
                                scps[:, hh * 128:(hh + 1) * 128],
                                qT[off:off + 64, g, b * 128:(b + 1) * 128],
                                kTt[off:off + 64, g, b * 128:(b + 1) * 128],
                                start=False, stop=(hh == 3), skip_group_check=True,
                            )
                    else:
                        for hh in range(4):
                            h = q4 * 4 + hh
                            g, off = h // 2, (h % 2) * 64
                            nc.tensor.matmul(
                                scps[:, hh * 128:(hh + 1) * 128],
                                qT[off:off + 64, g, b * 128:(b + 1) * 128],
                                kTt[off:off + 64, g, b * 128:(b + 1) * 128],
                                start=True, stop=False,
                            )
                            nc.tensor.matmul(
                                scps[:, hh * 128:(hh + 1) * 128],
                                eye_sb[:], maskp4_sb[:, hh * 128:(hh + 1) * 128],
                                start=False, stop=True,
                            )
                    st[b].append(scps)

            def attn_soft(b, vtok, hT, st):
                E = sb.tile([128, H, 128], f16, tag="E", bufs=3, name=f"E{b}")
                ssum = sb.tile([128, H], f32, tag="ssum", bufs=4, name=f"ssum{b}")
                r = sb.tile([128, H], f32, tag="r", bufs=4, name=f"r{b}")
                if U_ACT_ACCUM:
                    for q4 in range(2):
                        scps = st[b][q4]
                        for hh in range(4):
                            h = q4 * 4 + hh
                            nc.scalar.activation(
                                E[:, h, :], scps[:, hh * 128:(hh + 1) * 128],
                                AF.Exp, scale=0.125, accum_out=ssum[:, h:h + 1],
                            )
                else:
                    for q4 in range(2):
                        scps = st[b][q4]
                        nc.scalar.activation(
                            E[:, q4 * 4:(q4 + 1) * 4, :], scps[:], AF.Exp,
                            scale=0.125,
                        )
                    nc.vector.tensor_reduce(ssum[:], E[:, :, :], AX.X, OP.add)
                nc.vector.reciprocal(r[:], ssum[:])
                at4 = []
                for q4 in range(2):
                    atps = psp.tile([128, 512], f32, tag="ps", name=f"atps{b}_{q4}")
                    for hh in range(4):
                        h = q4 * 4 + hh
                        diag = sb.tile([128, 128], f16, tag="diag", bufs=6,
                                       name=f"diag{b}_{h}")
                        if h % 2 == 1 and U_IDENT:
                            nc.scalar.activation(
                                diag[:], eye_sb[:], AF.Identity, scale=r[:, h:h + 1]
                            )
                        else:
                            nc.vector.tensor_scalar(
                                diag[:], eye_sb[:], r[:, h:h + 1], None, OP.mult
                            )
                        nc.tensor.matmul(
                            atps[:, hh * 128:(hh + 1) * 128],
                            E[:, h, :], diag[:], start=True, stop=True,
                        )
                    at = sb.tile([128, 512], f16, tag="at", bufs=4,
                                 name=f"at{b}_{q4}")
                    if q4 == 1 and U_ACT_EVAC:
                        nc.scalar.activation(at[:], atps[:], AF.Copy)
                    else:
                        nc.vector.tensor_copy(at[:], atps[:])
                    at4.append(at)
                hps = psp.tile([128, 512], f32, tag="ps", name=f"hps{b}")
                for h in range(H):
                    g, off = h // 2, (h % 2) * 64
                    nc.tensor.matmul(
                        hps[off:off + 64, g * 128:(g + 1) * 128],
                        vtok[:, b, h * 64:(h + 1) * 64],
                        at4[h // 4][:, (h % 4) * 128:(h % 4 + 1) * 128],
                        start=True, stop=True, tile_position=(0, off),
                    )
                if U_HPS1:
                    nc.vector.tensor_copy(
                        hT[:, :, b * 128:(b + 1) * 128],
                        hps.rearrange("p (g t) -> p g t", g=KT),
                    )
                else:
                    for g in range(KT):
                        nc.vector.tensor_copy(
                            hT[:, g, b * 128:(b + 1) * 128],
                            hps[:, g * 128:(g + 1) * 128],
                        )

            def outproj_chunk(li, c, w4, hT, asum, asq):
                wc_sb = w4[3]
                sl = slice(c * 512, (c + 1) * 512)
                for m in range(KT):
                    cps = psp.tile([128, 512], f32, tag="ps", name="cps")
                    for k in range(KT):
                        nc.tensor.matmul(
                            cps[:], wc_sb[:, k, m * 128:(m + 1) * 128],
                            hT[:, k, sl], start=(k == 0), stop=(k == KT - 1),
                        )
                    nc.vector.scalar_tensor_tensor(
                        xb[m][:, sl], cps[:], 1.0, xb[m][:, sl],
                        OP.mult, OP.add, accum_out=asum[:, m, c:c + 1],
                    )
                    scrap = sb.tile([128, 512], f16, tag="scrap", bufs=2,
                                    name="scrap")
                    if U_ACT_ACCUM:
                        nc.scalar.activation(
                            scrap[:], xb[m][:, sl], AF.Square,
                            accum_out=asq[:, m, c:c + 1],
                        )
                    else:
                        nc.vector.scalar_tensor_tensor(
                            scrap[:], xb[m][:, sl], 1.0, xb[m][:, sl],
                            OP.mult, OP.mult, accum_out=asq[:, m, c:c + 1],
                        )

            def ffn_chunk(li, c, w1_sb, w2_sb, asum, asq):
                sl = slice(c * 512, (c + 1) * 512)
                midt = sb.tile([128, MKT, 512], f16, tag="mid", bufs=1,
                               name=f"mid{li}_{c}")
                for mm in range(MKT):
                    mps = psp.tile([128, 512], f32, tag="ps", name="mps")
                    for k in range(KT):
                        nc.tensor.matmul(
                            mps[:], w1_sb[:, k, mm * 128:(mm + 1) * 128],
                            xb[k][:, sl], start=(k == 0), stop=(k == KT - 1),
                        )
                    if U_ACT_EVAC:
                        nc.scalar.activation(midt[:, mm, :], mps[:], AF.Relu)
                    else:
                        nc.vector.tensor_scalar(
                            midt[:, mm, :], mps[:], 0.0, None, OP.max
                        )
                for m in range(KT):
                    ops = psp.tile([128, 512], f32, tag="ps", name="ops")
                    for k in range(MKT):
                        nc.tensor.matmul(
                            ops[:], w2_sb[:, k, m * 128:(m + 1) * 128],
                            midt[:, k, :], start=(k == 0), stop=(k == MKT - 1),
                        )
                    nc.vector.scalar_tensor_tensor(
                        xb[m][:, sl], ops[:], 1.0, xb[m][:, sl],
                        OP.mult, OP.add, accum_out=asum[:, m, c:c + 1],
                    )
                    scrap = sb.tile([128, 512], f16, tag="scrap", bufs=2,
                                    name="scrap")
                    if U_ACT_ACCUM:
                        nc.scalar.activation(
                            scrap[:], xb[m][:, sl], AF.Square,
                            accum_out=asq[:, m, c:c + 1],
                        )
                    else:
                        nc.vector.scalar_tensor_tensor(
                            scrap[:], xb[m][:, sl], 1.0, xb[m][:, sl],
                            OP.mult, OP.mult, accum_out=asq[:, m, c:c + 1],
                        )

            def bn(li, bnidx, asum, asq, last=False):
                red = sb.tile([128, 2 * KT], f32, tag="red", bufs=2,
                              name=f"red{li}_{bnidx}")
                for m in range(KT):
                    nc.vector.tensor_reduce(red[:, 2 * m:2 * m + 1],
                                            asum[:, m, :], AX.X, OP.add)
                    nc.vector.tensor_reduce(red[:, 2 * m + 1:2 * m + 2],
                                            asq[:, m, :], AX.X, OP.add)
                cin = dram.tile([128, 2 * KT], f32, tag="cin",
                                name=f"cin{li}_{bnidx}")
                cout = dram.tile([128, 2 * KT], f32, tag="cout",
                                 name=f"cout{li}_{bnidx}")
                nc.sync.dma_start(cin[:], red[:])
                nc.gpsimd.collective_compute(
                    "AllReduce", OP.add,
                    replica_groups=[list(range(NCORES))],
                    ins=[cin.opt()], outs=[cout.opt()],
                )
                # keep the PE HAM-warm while the allreduce is in flight
                for dnum in range(NDUMMY):
                    dps = psp.tile([128, 512], f32, tag="ps",
                                   name=f"dps{li}_{bnidx}_{dnum}")
                    nc.tensor.matmul(dps[:], eye_sb[:], maskp4_sb[:],
                                     start=True, stop=True)
                redg = sb.tile([128, 2 * KT], f32, tag="redg", bufs=2,
                               name=f"redg{li}_{bnidx}")
                nc.sync.dma_start(redg[:], cout[:])
                redv = redg.rearrange("p (m two) -> p m two", two=2)
                stat = sb.tile([128, 4, KT], f32, tag="stat", bufs=2,
                               name=f"stat{li}_{bnidx}")
                a_sb = sb.tile([128, KT], f32, tag="a_sb", bufs=2,
                               name=f"a{li}_{bnidx}")
                bneg = sb.tile([128, KT], f32, tag="bneg", bufs=2,
                               name=f"bneg{li}_{bnidx}")
                nc.vector.tensor_scalar(stat[:, 0, :], redv[:, :, 0],
                                        1.0 / NTOT, None, OP.mult)
                nc.vector.tensor_scalar(stat[:, 1, :], redv[:, :, 1],
                                        1.0 / NTOT, None, OP.mult)
                nc.vector.tensor_mul(stat[:, 2, :], stat[:, 0, :], stat[:, 0, :])
                nc.vector.tensor_sub(stat[:, 3, :], stat[:, 1, :], stat[:, 2, :])
                nc.vector.tensor_scalar(stat[:, 3, :], stat[:, 3, :], EPS,
                                        None, OP.add)
                if U_LNEXP:
                    # a = 1/sqrt(var) = exp(-0.5*ln(var)); ln+exp share one ACT
                    # table set with the attention Exp -> no table reloads.
                    nc.scalar.activation(stat[:, 2, :], stat[:, 3, :], AF.Ln)
                    nc.scalar.activation(a_sb[:], stat[:, 2, :], AF.Exp, scale=-0.5)
                else:
                    nc.scalar.activation(stat[:, 2, :], stat[:, 3, :], AF.Sqrt)
                    nc.vector.reciprocal(a_sb[:], stat[:, 2, :])
                nc.vector.scalar_tensor_tensor(bneg[:], stat[:, 0, :], -1.0,
                                               a_sb[:], OP.mult, OP.mult)
                if last:
                    outst = []
                    for m in range(KT):
                        ot = sb.tile([128, T // 2], f32, tag=f"outst{m}",
                                     name=f"outst{m}")
                        outst.append(ot)
                    for c in range(NCH):
                        for m in range(KT):
                            src = xv[m][:, 4 * c:4 * c + 4, :, 0, :]
                            dst = outst[m].rearrange(
                                "p (b a t) -> p b a t", b=BL, a=A
                            )[:, 4 * c:4 * c + 4, :, :]
                            if m % 2 == 1 and U_IDENT:
                                nc.scalar.activation(
                                    dst, src, AF.Identity,
                                    bias=bneg[:, m:m + 1], scale=a_sb[:, m:m + 1],
                                )
                            else:
                                nc.vector.tensor_scalar(
                                    dst, src, a_sb[:, m:m + 1],
                                    bneg[:, m:m + 1], OP.mult, OP.add,
                                )
                    for m in range(KT):
                        nc.sync.dma_start(out_d[m * 128:(m + 1) * 128, :],
                                          outst[m][:])
                else:
                    for c in range(NCH):
                        for m in range(KT):
                            sl = slice(c * 512, (c + 1) * 512)
                            if m % 2 == 1 and U_IDENT:
                                nc.scalar.activation(
                                    xb[m][:, sl], xb[m][:, sl], AF.Identity,
                                    bias=bneg[:, m:m + 1], scale=a_sb[:, m:m + 1],
                                )
                            else:
                                nc.vector.tensor_scalar(
                                    xb[m][:, sl], xb[m][:, sl],
                                    a_sb[:, m:m + 1], bneg[:, m:m + 1],
                                    OP.mult, OP.add,
                                )

            # ---- layers ----
            for li in range(NLAYERS):
                w4 = wqkv_next
                w1_sb, w2_sb = alloc_w12(li)
                qT = sb.tile([128, KT, T], f16, tag="qT", name=f"qT{li}")
                kTt = sb.tile([128, KT, T], f16, tag="kT", name=f"kT{li}")
                vtok = sb.tile([128, BL, D], f16, tag="vtok", name=f"vtok{li}")
                hT = sb.tile([128, KT, T], f16, tag="hT", name=f"hT{li}")
                asum1 = sb.tile([128, KT, NCH], f32, tag="asum", bufs=2,
                                name=f"asum1_{li}")
                asq1 = sb.tile([128, KT, NCH], f32, tag="asq", bufs=2,
                               name=f"asq1_{li}")
                asum2 = sb.tile([128, KT, NCH], f32, tag="asum", bufs=2,
                                name=f"asum2_{li}")
                asq2 = sb.tile([128, KT, NCH], f32, tag="asq", bufs=2,
                               name=f"asq2_{li}")

                st = {}
                qkv_chunk(li, 0, w4, qT, kTt, vtok)
                qkv_chunk(li, 1, w4, qT, kTt, vtok)
                attn_scores(0, qT, kTt, st)
                for b in range(BL):
                    if b < BL - 1:
                        attn_scores(b + 1, qT, kTt, st)
                    attn_soft(b, vtok, hT, st)
                    if b == 3:
                        qkv_chunk(li, 2, w4, qT, kTt, vtok)
                    elif b == 7:
                        qkv_chunk(li, 3, w4, qT, kTt, vtok)
                    elif b == 11:
                        outproj_chunk(li, 0, w4, hT, asum1, asq1)
                outproj_chunk(li, 1, w4, hT, asum1, asq1)
                outproj_chunk(li, 2, w4, hT, asum1, asq1)
                outproj_chunk(li, 3, w4, hT, asum1, asq1)
                if li + 1 < NLAYERS:
                    wqkv_next = alloc_wqkv(li + 1)
                bn(li, 1, asum1, asq1)
                for c in range(NCH):
                    ffn_chunk(li, c, w1_sb, w2_sb, asum2, asq2)
                bn(li, 2, asum2, asq2, last=(li == NLAYERS - 1))
    return nc


def _prep_inputs(inputs):
    """Host-side sharding/layout prep. Returns per-core in_maps."""
    obs = np.asarray(inputs["obs_emb"], np.float32)        # [L,B,A,D]
    onehot = np.asarray(inputs["act_onehot"], np.float32)  # [L,B,A,ACTN]
    actW = np.asarray(inputs["act_W"], np.float32)         # [ACTN,D]
    pos = np.asarray(inputs["pos"], np.float32)            # [L,D]
    seg = np.asarray(inputs["seg_emb"], np.float32)        # [A,D]
    wq = np.ascontiguousarray(np.asarray(inputs["Wq"], np.float32)).astype(np.float16)
    wk = np.ascontiguousarray(np.asarray(inputs["Wk"], np.float32)).astype(np.float16)
    wv = np.ascontiguousarray(np.asarray(inputs["Wv"], np.float32)).astype(np.float16)
    wc = np.ascontiguousarray(np.asarray(inputs["Wc"], np.float32)).astype(np.float16)
    w1 = np.ascontiguousarray(np.asarray(inputs["W1"], np.float32)).astype(np.float16)
    w2 = np.ascontiguousarray(np.asarray(inputs["W2"], np.float32)).astype(np.float16)
    mask = np.asarray(inputs["mask"])                      # [F,F] bool

    eye = np.eye(128, dtype=np.float32).astype(np.float16)
    # permute mask from reference order (a*32 + 2t + s) to ours (a*32 + s*16 + t)
    perm = np.array([a * 32 + 2 * t + s
                     for a in range(A) for s in range(2) for t in range(L)])
    mp = mask[perm][:, perm]
    maskp = np.where(mp, 0.0, MASKNEG).astype(np.float32)
    maskp4 = np.ascontiguousarray(
        np.concatenate([maskp] * 4, axis=1)).astype(np.float16)

    # augmented act weights: [actW; pos; seg] so the act matmul adds pos+seg
    actW36 = np.concatenate(
        [actW, pos, seg, np.zeros((AUGP - AUG, D), np.float32)],
        axis=0).astype(np.float16)
    # indicator rows for act tokens in (b, a, t) order
    idx = np.arange(T // 2)
    t_of, a_of = idx % L, (idx // L) % A
    tind = (np.arange(L)[:, None] == t_of[None, :]).astype(np.float32)
    aind = (np.arange(A)[:, None] == a_of[None, :]).astype(np.float32)

    obsb = obs + pos[:, None, None, :] + seg[None, None, :, :]  # [L,B,A,D]

    in_maps = []
    for c in range(NCORES):
        bs = slice(c * BL, (c + 1) * BL)
        o = obsb[:, bs].transpose(3, 1, 2, 0)                  # [D,BL,A,L]
        xinit = np.zeros((D, BL, A, 2, L), np.float32)
        xinit[:, :, :, 0, :] = o
        xinit = np.ascontiguousarray(
            xinit.reshape(D, T)).astype(np.float16)
        oh = onehot[:, bs].transpose(3, 1, 2, 0).reshape(ACTN, T // 2)
        onehot36 = np.ascontiguousarray(np.concatenate(
            [oh, tind, aind, np.zeros((AUGP - AUG, T // 2), np.float32)],
            axis=0)).astype(np.float16)
        in_maps.append({
            "xinit": xinit, "onehot36": onehot36, "actW36": actW36,
            "wq": wq, "wk": wk, "wv": wv, "wc": wc, "w1": w1, "w2": w2,
            "eye": eye, "maskp4": maskp4,
        })
    return in_maps


def run_impl(inputs, trace=False):
    in_maps = _prep_inputs(inputs)
    nc = build_nc()
    nc.compile()
    res = run_bass_kernel_spmd(nc, in_maps, list(range(NCORES)), trace=trace)
    outs = []
    for c in range(NCORES):
        o = res.results[c]["out"]                     # [512, 1024]
        outs.append(o.reshape(D, BL, 2 * L * A // 2).transpose(1, 2, 0))
    full = np.concatenate(outs, axis=0)               # [B, 64, 512]
    return np.ascontiguousarray(full.astype(np.float32)), res


def kernel(**inputs) -> np.ndarray:
    out, _ = run_impl(inputs, trace=False)
    return out


# revision 12
# speedup vs baseline: 1.0169x; 1.0014x over previous
"""Trainium2 Bass kernel for nn_JointPredReprModule (4-layer transformer w/ BatchNorm).

Sharding: data-parallel over batch (128 -> 16 per core x 8 cores).
Per-core activations are feature-major: x[d, token], token = b*128 + a*32 + s*16 + t
(s=0 obs slot, s=1 act slot; reference order is a*32 + 2t + s — mask is permuted to match).

v2 design notes (vs the earlier baseline):
- Residual stream kept in fp16 only (no fp32 shadow; fp16 matmuls run at the
  same 1 cycle/row as bf16 but with 8x finer mantissa — 4-layer rel err 2.6e-3
  vs 1.8e-2 with bf16). BN stats accumulate in fp32.
- pos+seg biases folded on the host: obs slots pre-biased in the xinit upload, act
  slots get bias via an augmented one-hot matmul (extra t/a indicator rows).
- Attention: per-head Exp on the scalar engine with accum_out giving the softmax
  denominators for free (no big vector reduce).
- PSUM evacuation split across Scalar (ACT copy/relu/square) and Vector engines.
- 1/sigma via exp(-0.5*ln(var)) so the scalar engine stays on one ACT table set
  (shared with the attention Exp) — no per-BN table reloads.
- Layer weights prefetched early (QKV double-buffered, FFN at layer start) so
  their DMA never collides with the BN stats collective.
- Output staged compactly (obs slots only) and DMA'd per feature tile.
"""

import os
import numpy as np
import ml_dtypes

import concourse.bass as bass
import concourse.bacc as bacc
import concourse.mybir as mybir
import concourse.tile as tile
from concourse.bass_utils import run_bass_kernel_spmd

f32 = mybir.dt.float32
f16 = mybir.dt.float16
AX = mybir.AxisListType
OP = mybir.AluOpType
AF = mybir.ActivationFunctionType

L, B, A, D, H, ACTN = 16, 128, 4, 512, 8, 16
F = 2 * L * A          # 128 tokens per batch element
NCORES = 8
BL = B // NCORES       # 16 batch elems per core
T = BL * F             # 2048 tokens per core
DH = D // H            # 64
KT = D // 128          # 4 feature tiles
NCH = T // 512         # 4 token chunks of 512
MID = 4 * D            # 2048
MKT = MID // 128       # 16
EPS = 1e-5
NLAYERS = int(os.environ.get("KERNEL_NLAYERS", "4"))
MASKNEG = -240.0       # pre-scale; exp scale is 1/8 -> -30 post-scale
NTOT = float(B * F)    # global BN sample count
AUG = ACTN + L + A     # augmented one-hot rows (act + t-indicator + a-indicator)
NDUMMY = int(os.environ.get("KERNEL_NDUMMY", "0"))
# risky-construct toggles (default = safe/baseline-style)
_on = lambda name, d: os.environ.get(name, d) == "1"
U_ACT_EVAC = _on("KERNEL_ACT_EVAC", "1")   # psum evac on scalar engine
U_IDENT = _on("KERNEL_IDENT", "1")         # ACT Identity with scale/bias APs
U_LNEXP = _on("KERNEL_LNEXP", "1")         # rsqrt via ln+exp (single ACT table)
# one N=512 mask MM per score bank: crashes the device (bad accumulation-group
# codegen with a full-bank opener + per-slot closers) — keep OFF
U_FASTMASK = _on("KERNEL_FASTMASK", "0")
U_HPS1 = _on("KERNEL_HPS1", "1")           # single rearranged hT copy
U_ACT_ACCUM = _on("KERNEL_ACT_ACCUM", "1") # accum_out on scalar activation
AUGP = 64                             # padded augmented one-hot rows


def build_nc():
    nc = bacc.Bacc(None, target_bir_lowering=False, debug=False, num_devices=NCORES)

    xinit_d = nc.dram_tensor("xinit", [D, T], f16, kind="ExternalInput")
    onehot_d = nc.dram_tensor("onehot36", [AUGP, T // 2], f16, kind="ExternalInput")
    actW_d = nc.dram_tensor("actW36", [AUGP, D], f16, kind="ExternalInput")
    wq_d = nc.dram_tensor("wq", [4, D, D], f16, kind="ExternalInput")
    wk_d = nc.dram_tensor("wk", [4, D, D], f16, kind="ExternalInput")
    wv_d = nc.dram_tensor("wv", [4, D, D], f16, kind="ExternalInput")
    wc_d = nc.dram_tensor("wc", [4, D, D], f16, kind="ExternalInput")
    w1_d = nc.dram_tensor("w1", [4, D, MID], f16, kind="ExternalInput")
    w2_d = nc.dram_tensor("w2", [4, MID, D], f16, kind="ExternalInput")
    eye_d = nc.dram_tensor("eye", [128, 128], f16, kind="ExternalInput")
    maskp4_d = nc.dram_tensor("maskp4", [128, 512], f16, kind="ExternalInput")
    out_d = nc.dram_tensor("out", [D, T // 2], f32, kind="ExternalOutput")

    with tile.TileContext(nc) as tc:
        with (
            tc.tile_pool(name="sb", bufs=1) as sb,
            tc.tile_pool(name="ps", bufs=8, space="PSUM") as psp,
            tc.tile_pool(name="dram", bufs=2, space="DRAM") as dram,
        ):
            eye_sb = sb.tile([128, 128], f16, tag="eye", name="eye_sb")
            maskp4_sb = sb.tile([128, 512], f16, tag="maskp4", name="maskp4_sb")
            actW_sb = sb.tile([AUGP, D], f16, tag="actW", name="actW_sb")
            onehot_sb = sb.tile([AUGP, T // 2], f16, tag="onehot", name="onehot_sb")
            nc.sync.dma_start(eye_sb[:], eye_d[:])
            nc.sync.dma_start(maskp4_sb[:], maskp4_d[:])
            nc.sync.dma_start(actW_sb[:], actW_d[:])
            nc.sync.dma_start(onehot_sb[:], onehot_d[:])

            # residual stream (bf16), token = b*128 + a*32 + s*16 + t
            xb = []
            for k in range(KT):
                xk = sb.tile([128, T], f16, tag=f"xb{k}", name=f"xb{k}")
                xb.append(xk)
            xv = [x.rearrange("p (b a s t) -> p b a s t", b=BL, a=A, s=2, t=L)
                  for x in xb]

            # ---- weight prefetch machinery ----
            def alloc_wqkv(li):
                wq_sb = sb.tile([128, KT, D], f16, tag="wq", bufs=2, name=f"wq{li}")
                wk_sb = sb.tile([128, KT, D], f16, tag="wk", bufs=2, name=f"wk{li}")
                wv_sb = sb.tile([128, KT, D], f16, tag="wv", bufs=2, name=f"wv{li}")
                wc_sb = sb.tile([128, KT, D], f16, tag="wc", bufs=2, name=f"wc{li}")
                nc.sync.dma_start(wq_sb[:], wq_d[li].rearrange("(k p) m -> p k m", p=128))
                nc.sync.dma_start(wk_sb[:], wk_d[li].rearrange("(k p) m -> p k m", p=128))
                nc.sync.dma_start(wv_sb[:], wv_d[li].rearrange("(k p) m -> p k m", p=128))
                nc.sync.dma_start(wc_sb[:], wc_d[li].rearrange("(k p) m -> p k m", p=128))
                return (wq_sb, wk_sb, wv_sb, wc_sb)

            def alloc_w12(li):
                w1_sb = sb.tile([128, KT, MID], f16, tag="w1", bufs=1, name=f"w1_{li}")
                w2_sb = sb.tile([128, MKT, D], f16, tag="w2", bufs=1, name=f"w2_{li}")
                nc.sync.dma_start(w1_sb[:], w1_d[li].rearrange("(k p) m -> p k m", p=128))
                nc.sync.dma_start(w2_sb[:], w2_d[li].rearrange("(k p) m -> p k m", p=128))
                return w1_sb, w2_sb

            wqkv_next = alloc_wqkv(0)

            # ---- embedding ----
            for k in range(KT):
                nc.sync.dma_start(xb[k][:], xinit_d[k * 128:(k + 1) * 128, :])
            # act embedding (+ pos/seg bias via indicator rows)
            for m in range(KT):
                for cc in range(2):
                    aps = psp.tile([128, 512], f32, tag="ps", name="aps")
                    nc.tensor.matmul(
                        aps[:],
                        actW_sb[:, m * 128:(m + 1) * 128],
                        onehot_sb[:, cc * 512:(cc + 1) * 512],
                        start=True, stop=True,
                    )
                    nc.vector.tensor_copy(
                        xv[m][:, 8 * cc: 8 * cc + 8, :, 1, :], aps[:]
                    )

            # ---- per-layer pieces ----
            def qkv_chunk(li, c, w4, qT, kTt, vtok):
                wq_sb, wk_sb, wv_sb, _ = w4
                sl = slice(c * 512, (c + 1) * 512)
                for m in range(KT):
                    qps = psp.tile([128, 512], f32, tag="ps", name="qps")
                    for k in range(KT):
                        nc.tensor.matmul(
                            qps[:], wq_sb[:, k, m * 128:(m + 1) * 128],
                            xb[k][:, sl], start=(k == 0), stop=(k == KT - 1),
                        )
                    if U_ACT_EVAC:
                        nc.scalar.activation(qT[:, m, sl], qps[:], AF.Copy)
                    else:
                        nc.vector.tensor_copy(qT[:, m, sl], qps[:])
                for m in range(KT):
                    kps = psp.tile([128, 512], f32, tag="ps", name="kps")
                    for k in range(KT):
                        nc.tensor.matmul(
                            kps[:], wk_sb[:, k, m * 128:(m + 1) * 128],
                            xb[k][:, sl], start=(k == 0), stop=(k == KT - 1),
                        )
                    if U_ACT_EVAC:
                        nc.scalar.activation(kTt[:, m, sl], kps[:], AF.Copy)
                    else:
                        nc.vector.tensor_copy(kTt[:, m, sl], kps[:])
                for tt in range(4 * c, 4 * c + 4):
                    vps = psp.tile([128, 512], f32, tag="ps", name="vps")
                    for k in range(KT):
                        nc.tensor.matmul(
                            vps[:], xb[k][:, tt * 128:(tt + 1) * 128],
                            wv_sb[:, k, :], start=(k == 0), stop=(k == KT - 1),
                        )
                    nc.vector.tensor_copy(vtok[:, tt, :], vps[:])

            def attn_scores(b, qT, kTt, st):
                st[b] = []
                for q4 in range(2):
                    scps = psp.tile([128, 512], f32, tag="ps", name=f"scps{b}_{q4}")
                    if U_FASTMASK:
                        # mask first: one N=512 MM opens the bank (clears
                        # has_written bank-wide), scores accumulate after
                        nc.tensor.matmul(
                            scps[:], eye_sb[:], maskp4_sb[:],
                            start=True, stop=False, skip_group_check=True,
                        )
                        for hh in range(4):
                            h = q4 * 4 + hh
                            g, off = h // 2, (h % 2) * 64
                            nc.tensor.matmul(
                                scps[:, hh * 128:(hh + 1) * 128],
                                qT[off:off + 64, g, b * 128:(b + 1) * 128],
                                kTt[off:off + 64, g, b * 128:(b + 1) * 128],
                                start=False, stop=(hh == 3), skip_group_check=True,
                            )
                    else:
                        for hh in range(4):
                            h = q4 * 4 + hh
                            g, off = h // 2, (h % 2) * 64
                            nc.tensor.matmul(
                                scps[:, hh * 128:(hh + 1) * 128],
                                qT[off:off + 64, g, b * 128:(b + 1) * 128],
                                kTt[off:off + 64, g, b * 128:(b + 1) * 128],
                                start=True, stop=False,
                            )
                            nc.tensor.matmul(
                                scps[:, hh * 128:(hh + 1) * 128],
                                eye_sb[:], maskp4_sb[:, hh * 128:(hh + 1) * 128],
                                start=False, stop=True,
                            )
                    st[b].append(scps)

            def attn_soft(b, vtok, hT, st):
                E = sb.tile([128, H, 128], f16, tag="E", bufs=3, name=f"E{b}")
                ssum = sb.tile([128, H], f32, tag="ssum", bufs=4, name=f"ssum{b}")
                r = sb.tile([128, H], f32, tag="r", bufs=4, name=f"r{b}")
                if U_ACT_ACCUM:
                    for q4 in range(2):
                        scps = st[b][q4]
                        for hh in range(4):
                            h = q4 * 4 + hh
                            nc.scalar.activation(
                                E[:, h, :], scps[:, hh * 128:(hh + 1) * 128],
                                AF.Exp, scale=0.125, accum_out=ssum[:, h:h + 1],
                            )
                else:
                    for q4 in range(2):
                        scps = st[b][q4]
                        nc.scalar.activation(
                            E[:, q4 * 4:(q4 + 1) * 4, :], scps[:], AF.Exp,
                            scale=0.125,
                        )
                    nc.vector.tensor_reduce(ssum[:], E[:, :, :], AX.X, OP.add)
                nc.vector.reciprocal(r[:], ssum[:])
                at4 = []
                for q4 in range(2):
                    atps = psp.tile([128, 512], f32, tag="ps", name=f"atps{b}_{q4}")
                    for hh in range(4):
                        h = q4 * 4 + hh
                        diag = sb.tile([128, 128], f16, tag="diag", bufs=6,
                                       name=f"diag{b}_{h}")
                        if h % 2 == 1 and U_IDENT:
                            nc.scalar.activation(
                                diag[:], eye_sb[:], AF.Identity, scale=r[:, h:h + 1]
                            )
                        else:
                            nc.vector.tensor_scalar(
                                diag[:], eye_sb[:], r[:, h:h + 1], None, OP.mult
                            )
                        nc.tensor.matmul(
                            atps[:, hh * 128:(hh + 1) * 128],
                            E[:, h, :], diag[:], start=True, stop=True,
                        )
                    at = sb.tile([128, 512], f16, tag="at", bufs=4,
                                 name=f"at{b}_{q4}")
                    if q4 == 1 and U_ACT_EVAC:
                        nc.scalar.activation(at[:], atps[:], AF.Copy)
                    else:
                        nc.vector.tensor_copy(at[:], atps[:])
                    at4.append(at)
                hps = psp.tile([128, 512], f32, tag="ps", name=f"hps{b}")
                for h in range(H):
                    g, off = h // 2, (h % 2) * 64
                    nc.tensor.matmul(
                        hps[off:off + 64, g * 128:(g + 1) * 128],
                        vtok[:, b, h * 64:(h + 1) * 64],
                        at4[h // 4][:, (h % 4) * 128:(h % 4 + 1) * 128],
                        start=True, stop=True, tile_position=(0, off),
                    )
                if U_HPS1:
                    nc.vector.tensor_copy(
                        hT[:, :, b * 128:(b + 1) * 128],
                        hps.rearrange("p (g t) -> p g t", g=KT),
                    )
                else:
                    for g in range(KT):
                        nc.vector.tensor_copy(
                            hT[:, g, b * 128:(b + 1) * 128],
                            hps[:, g * 128:(g + 1) * 128],
                        )

            def outproj_chunk(li, c, w4, hT, asum, asq):
                wc_sb = w4[3]
                sl = slice(c * 512, (c + 1) * 512)
                for m in range(KT):
                    cps = psp.tile([128, 512], f32, tag="ps", name="cps")
                    for k in range(KT):
                        nc.tensor.matmul(
                            cps[:], wc_sb[:, k, m * 128:(m + 1) * 128],
                            hT[:, k, sl], start=(k == 0), stop=(k == KT - 1),
                        )
                    nc.vector.scalar_tensor_tensor(
                        xb[m][:, sl], cps[:], 1.0, xb[m][:, sl],
                        OP.mult, OP.add, accum_out=asum[:, m, c:c + 1],
                    )
                    scrap = sb.tile([128, 512], f16, tag="scrap", bufs=2,
                                    name="scrap")
                    if U_ACT_ACCUM:
                        nc.scalar.activation(
                            scrap[:], xb[m][:, sl], AF.Square,
                            accum_out=asq[:, m, c:c + 1],
                        )
                    else:
                        nc.vector.scalar_tensor_tensor(
                            scrap[:], xb[m][:, sl], 1.0, xb[m][:, sl],
                            OP.mult, OP.mult, accum_out=asq[:, m, c:c + 1],
                        )

            def ffn_chunk(li, c, w1_sb, w2_sb, asum, asq):
                sl = slice(c * 512, (c + 1) * 512)
                midt = sb.tile([128, MKT, 512], f16, tag="mid", bufs=1,
                               name=f"mid{li}_{c}")
                for mm in range(MKT):
                    mps = psp.tile([128, 512], f32, tag="ps", name="mps")
                    for k in range(KT):
                        nc.tensor.matmul(
                            mps[:], w1_sb[:, k, mm * 128:(mm + 1) * 128],
                            xb[k][:, sl], start=(k == 0), stop=(k == KT - 1),
                        )
                    if U_ACT_EVAC:
                        nc.scalar.activation(midt[:, mm, :], mps[:], AF.Relu)
                    else:
                        nc.vector.tensor_scalar(
                            midt[:, mm, :], mps[:], 0.0, None, OP.max
                        )
                for m in range(KT):
                    ops = psp.tile([128, 512], f32, tag="ps", name="ops")
                    for k in range(MKT):
                        nc.tensor.matmul(
                            ops[:], w2_sb[:, k, m * 128:(m + 1) * 128],
                            midt[:, k, :], start=(k == 0), stop=(k == MKT - 1),
                        )
                    nc.vector.scalar_tensor_tensor(
                        xb[m][:, sl], ops[:], 1.0, xb[m][:, sl],
                        OP.mult, OP.add, accum_out=asum[:, m, c:c + 1],
                    )
                    scrap = sb.tile([128, 512], f16, tag="scrap", bufs=2,
                                    name="scrap")
                    if U_ACT_ACCUM:
                        nc.scalar.activation(
                            scrap[:], xb[m][:, sl], AF.Square,
                            accum_out=asq[:, m, c:c + 1],
                        )
                    else:
                        nc.vector.scalar_tensor_tensor(
                            scrap[:], xb[m][:, sl], 1.0, xb[m][:, sl],
                            OP.mult, OP.mult, accum_out=asq[:, m, c:c + 1],
                        )

            def bn(li, bnidx, asum, asq, last=False):
                red = sb.tile([128, 2 * KT], f32, tag="red", bufs=2,
                              name=f"red{li}_{bnidx}")
                for m in range(KT):
                    nc.vector.tensor_reduce(red[:, 2 * m:2 * m + 1],
                                            asum[:, m, :], AX.X, OP.add)
                    nc.vector.tensor_reduce(red[:, 2 * m + 1:2 * m + 2],
                                            asq[:, m, :], AX.X, OP.add)
                cin = dram.tile([128, 2 * KT], f32, tag="cin",
                                name=f"cin{li}_{bnidx}")
                cout = dram.tile([128, 2 * KT], f32, tag="cout",
                                 name=f"cout{li}_{bnidx}")
                nc.sync.dma_start(cin[:], red[:])
                nc.gpsimd.collective_compute(
                    "AllReduce", OP.add,
                    replica_groups=[list(range(NCORES))],
                    ins=[cin.opt()], outs=[cout.opt()],
                )
                # keep the PE HAM-warm while the allreduce is in flight
                for dnum in range(NDUMMY):
                    dps = psp.tile([128, 512], f32, tag="ps",
                                   name=f"dps{li}_{bnidx}_{dnum}")
                    nc.tensor.matmul(dps[:], eye_sb[:], maskp4_sb[:],
                                     start=True, stop=True)
                redg = sb.tile([128, 2 * KT], f32, tag="redg", bufs=2,
                               name=f"redg{li}_{bnidx}")
                nc.sync.dma_start(redg[:], cout[:])
                redv = redg.rearrange("p (m two) -> p m two", two=2)
                stat = sb.tile([128, 4, KT], f32, tag="stat", bufs=2,
                               name=f"stat{li}_{bnidx}")
                a_sb = sb.tile([128, KT], f32, tag="a_sb", bufs=2,
                               name=f"a{li}_{bnidx}")
                bneg = sb.tile([128, KT], f32, tag="bneg", bufs=2,
                               name=f"bneg{li}_{bnidx}")
                nc.vector.tensor_scalar(stat[:, 0, :], redv[:, :, 0],
                                        1.0 / NTOT, None, OP.mult)
                nc.vector.tensor_scalar(stat[:, 1, :], redv[:, :, 1],
                                        1.0 / NTOT, None, OP.mult)
                nc.vector.tensor_mul(stat[:, 2, :], stat[:, 0, :], stat[:, 0, :])
                nc.vector.tensor_sub(stat[:, 3, :], stat[:, 1, :], stat[:, 2, :])
                nc.vector.tensor_scalar(stat[:, 3, :], stat[:, 3, :], EPS,
                                        None, OP.add)
                if U_LNEXP:
                    # a = 1/sqrt(var) = exp(-0.5*ln(var)); ln+exp share one ACT
                    # table set with the attention Exp -> no table reloads.
                    nc.scalar.activation(stat[:, 2, :], stat[:, 3, :], AF.Ln)
                    nc.scalar.activation(a_sb[:], stat[:, 2, :], AF.Exp, scale=-0.5)
                else:
                    nc.scalar.activation(stat[:, 2, :], stat[:, 3, :], AF.Sqrt)
                    nc.vector.reciprocal(a_sb[:], stat[:, 2, :])
                nc.vector.scalar_tensor_tensor(bneg[:], stat[:, 0, :], -1.0,
                                               a_sb[:], OP.mult, OP.mult)
                if last:
                    outst = []
                    for m in range(KT):
                        ot = sb.tile([128, T // 2], f32, tag=f"outst{m}",
                                     name=f"outst{m}")
                        outst.append(ot)
                    for c in range(NCH):
                        for m in range(KT):
                            src = xv[m][:, 4 * c:4 * c + 4, :, 0, :]
                            dst = outst[m].rearrange(
                                "p (b a t) -> p b a t", b=BL, a=A
                            )[:, 4 * c:4 * c + 4, :, :]
                            if m % 2 == 1 and U_IDENT:
                                nc.scalar.activation(
                                    dst, src, AF.Identity,
                                    bias=bneg[:, m:m + 1], scale=a_sb[:, m:m + 1],
                                )
                            else:
                                nc.vector.tensor_scalar(
                                    dst, src, a_sb[:, m:m + 1],
                                    bneg[:, m:m + 1], OP.mult, OP.add,
                                )
                    for m in range(KT):
                        nc.sync.dma_start(out_d[m * 128:(m + 1) * 128, :],
                                          outst[m][:])
                else:
                    for c in range(NCH):
                        for m in range(KT):
                            sl = slice(c * 512, (c + 1) * 512)
                            if m % 2 == 1 and U_IDENT:
                                nc.scalar.activation(
                                    xb[m][:, sl], xb[m][:, sl], AF.Identity,
                                    bias=bneg[:, m:m + 1], scale=a_sb[:, m:m + 1],
                                )
                            else:
                                nc.vector.tensor_scalar(
                                    xb[m][:, sl], xb[m][:, sl],
                                    a_sb[:, m:m + 1], bneg[:, m:m + 1],
                                    OP.mult, OP.add,
                                )

            # ---- layers ----
            for li in range(NLAYERS):
                w4 = wqkv_next
                w1_sb, w2_sb = alloc_w12(li)
                qT = sb.tile([128, KT, T], f16, tag="qT", name=f"qT{li}")
                kTt = sb.tile([128, KT, T], f16, tag="kT", name=f"kT{li}")
                vtok = sb.tile([128, BL, D], f16, tag="vtok", name=f"vtok{li}")
                hT = sb.tile([128, KT, T], f16, tag="hT", name=f"hT{li}")
                asum1 = sb.tile([128, KT, NCH], f32, tag="asum", bufs=2,
                                name=f"asum1_{li}")
                asq1 = sb.tile([128, KT, NCH], f32, tag="asq", bufs=2,
                               name=f"asq1_{li}")
                asum2 = sb.tile([128, KT, NCH], f32, tag="asum", bufs=2,
                                name=f"asum2_{li}")
                asq2 = sb.tile([128, KT, NCH], f32, tag="asq", bufs=2,
                               name=f"asq2_{li}")

                st = {}
                qkv_chunk(li, 0, w4, qT, kTt, vtok)
                qkv_chunk(li, 1, w4, qT, kTt, vtok)
                attn_scores(0, qT, kTt, st)
                for b in range(BL):
                    if b < BL - 1:
                        attn_scores(b + 1, qT, kTt, st)
                    attn_soft(b, vtok, hT, st)
                    if b == 3:
                        qkv_chunk(li, 2, w4, qT, kTt, vtok)
                    elif b == 7:
                        qkv_chunk(li, 3, w4, qT, kTt, vtok)
                    elif b == 11:
                        outproj_chunk(li, 0, w4, hT, asum1, asq1)
                outproj_chunk(li, 1, w4, hT, asum1, asq1)
                outproj_chunk(li, 2, w4, hT, asum1, asq1)
                outproj_chunk(li, 3, w4, hT, asum1, asq1)
                if li + 1 < NLAYERS:
                    wqkv_next = alloc_wqkv(li + 1)
                bn(li, 1, asum1, asq1)
                for c in range(NCH):
                    ffn_chunk(li, c, w1_sb, w2_sb, asum2, asq2)
                bn(li, 2, asum2, asq2, last=(li == NLAYERS - 1))
    return nc


def _prep_inputs(inputs):
    """Host-side sharding/layout prep. Returns per-core in_maps."""
    obs = np.asarray(inputs["obs_emb"], np.float32)        # [L,B,A,D]
    onehot = np.asarray(inputs["act_onehot"], np.float32)  # [L,B,A,ACTN]
    actW = np.asarray(inputs["act_W"], np.float32)         # [ACTN,D]
    pos = np.asarray(inputs["pos"], np.float32)            # [L,D]
    seg = np.asarray(inputs["seg_emb"], np.float32)        # [A,D]
    wq = np.ascontiguousarray(np.asarray(inputs["Wq"], np.float32)).astype(np.float16)
    wk = np.ascontiguousarray(np.asarray(inputs["Wk"], np.float32)).astype(np.float16)
    wv = np.ascontiguousarray(np.asarray(inputs["Wv"], np.float32)).astype(np.float16)
    wc = np.ascontiguousarray(np.asarray(inputs["Wc"], np.float32)).astype(np.float16)
    w1 = np.ascontiguousarray(np.asarray(inputs["W1"], np.float32)).astype(np.float16)
    w2 = np.ascontiguousarray(np.asarray(inputs["W2"], np.float32)).astype(np.float16)
    mask = np.asarray(inputs["mask"])                      # [F,F] bool

    eye = np.eye(128, dtype=np.float32).astype(np.float16)
    # permute mask from reference order (a*32 + 2t + s) to ours (a*32 + s*16 + t)
    perm = np.array([a * 32 + 2 * t + s
                     for a in range(A) for s in range(2) for t in range(L)])
    mp = mask[perm][:, perm]
    maskp = np.where(mp, 0.0, MASKNEG).astype(np.float32)
    maskp4 = np.ascontiguousarray(
        np.concatenate([maskp] * 4, axis=1)).astype(np.float16)

    # augmented act weights: [actW; pos; seg] so the act matmul adds pos+seg
    actW36 = np.concatenate(
        [actW, pos, seg, np.zeros((AUGP - AUG, D), np.float32)],
        axis=0).astype(np.float16)
    # indicator rows for act tokens in (b, a, t) order
    idx = np.arange(T // 2)
    t_of, a_of = idx % L, (idx // L) % A
    tind = (np.arange(L)[:, None] == t_of[None, :]).astype(np.float32)
    aind = (np.arange(A)[:, None] == a_of[None, :]).astype(np.float32)

    obsb = obs + pos[:, None, None, :] + seg[None, None, :, :]  # [L,B,A,D]

    in_maps = []
    for c in range(NCORES):
        bs = slice(c * BL, (c + 1) * BL)
        o = obsb[:, bs].transpose(3, 1, 2, 0)                  # [D,BL,A,L]
        xinit = np.zeros((D, BL, A, 2, L), np.float32)
        xinit[:, :, :, 0, :] = o
        xinit = np.ascontiguousarray(
            xinit.reshape(D, T)).astype(np.float16)
        oh = onehot[:, bs].transpose(3, 1, 2, 0).reshape(ACTN, T // 2)
        onehot36 = np.ascontiguousarray(np.concatenate(
            [oh, tind, aind, np.zeros((AUGP - AUG, T // 2), np.float32)],
            axis=0)).astype(np.float16)
        in_maps.append({
            "xinit": xinit, "onehot36": onehot36, "actW36": actW36,
            "wq": wq, "wk": wk, "wv": wv, "wc": wc, "w1": w1, "w2": w2,
            "eye": eye, "maskp4": maskp4,
        })
    return in_maps


def run_impl(inputs, trace=False):
    in_maps = _prep_inputs(inputs)
    nc = build_nc()
    nc.compile()
    res = run_bass_kernel_spmd(nc, in_maps, list(range(NCORES)), trace=trace)
    outs = []
    for c in range(NCORES):
        o = res.results[c]["out"]                     # [512, 1024]
        outs.append(o.reshape(D, BL, 2 * L * A // 2).transpose(1, 2, 0))
    full = np.concatenate(outs, axis=0)               # [B, 64, 512]
    return np.ascontiguousarray(full.astype(np.float32)), res


def kernel(**inputs) -> np.ndarray:
    out, _ = run_impl(inputs, trace=False)
    return out
